# revision 7
# baseline (speedup 1.0000x reference)
"""Causal multi-head attention (B=32,T=512,C=1024,H=16,D=64) on 8 TRN2 cores.

Strategy: pure data-parallel over the batch axis (4 batches per core, no
collectives). Per core, per batch:
  - x^T [C,T] arrives pre-transposed from the host (layout prep only).
  - Q^T [HD,T] and V [T,HD] computed with bf16 matmuls (fp32 PSUM); K^T is
    written into per-head zero-padded [128,T] tiles so every PE matmul runs
    in the full 128x128 array mode (no tiling-mode switches/drains).
  - scores^T [s,t] blocks computed directly on PE (only the causal lower
    triangle of [T,T], packed into a [128,1280] PSUM tile per head).
  - softmax without max-subtraction: scores here are bounded (|s|<~3) so
    exp is safe in fp32; masked entries are zeroed by multiplying the
    exp'd diagonal blocks with a 0/1 triangular mask (gpsimd).
  - attn@V with a ones-augmented V column producing the softmax row-sums
    in the same matmul; all four t-chunk AV groups accumulate into ONE
    psum bank [128,4,128]; merged reciprocal + per-partition scales.
  - head-concat transpose via one batched DMA-transpose per t-chunk
    (issued from the sync queue); final projection with bias added during
    PSUM evacuation; fp32 output.

Pipelining (v2): the attention phase of batch b is engine-cadence bound
(exp on scalar ~1.33us/head, masks on gpsimd ~1.7us/head) while the PE
only has ~1.1us/head of matmul work.  To keep the PE saturated, batch
b+1's 24 QKV psum-chains are emitted as FILLER between head iterations
of batch b's attention and between proj chains, so the in-order PE queue
always has independent work while exp/mask/normalize latencies resolve.
Weights are DMA'd in m-major order so the first Q chain of batch 0 can
start after ~400KB instead of ~8MB.
"""

import sys

if "/opt/trn_rl_repo" not in sys.path:
    sys.path.insert(0, "/opt/trn_rl_repo")

import numpy as np
import ml_dtypes

B, T, C = 32, 512, 1024
H, D = 16, 64
HD = H * D
NCORES = 8
B_LOC = B // NCORES

_CACHE = {}


def build_nc(b_loc=B_LOC):
    import concourse.mybir as mybir
    from concourse import bacc
    from concourse.bass import ds, ts
    from concourse.tile import TileContext

    f32 = mybir.dt.float32
    bf16 = mybir.dt.bfloat16
    AF = mybir.ActivationFunctionType

    KO = C // 128  # 8 contraction chunks
    MO = HD // 128  # 8 output-row chunks
    TCH = T // 128  # 4 t-chunks
    SCALE = 1.0 / float(np.sqrt(C))

    # scores^T causal packing: s-chunk j covers t in [128j, T), width T-128j.
    # Packed into one PSUM tile [128, 1280] so no matmul output crosses a
    # 2KB bank boundary: j0@[0,512) bank0, j1@[512,896) bank1,
    # j3@[896,1024) bank1, j2@[1024,1280) bank2.
    widths = [T - 128 * j for j in range(TCH)]
    off = [0, 512, 1024, 896]
    PACK = 1280

    nc = bacc.Bacc("TRN2", target_bir_lowering=False)
    xT = nc.dram_tensor("xT", [b_loc, C, T], bf16, kind="ExternalInput")
    # m-major weight layouts: [MO, C, 128] so column-chunk m is contiguous
    wqm = nc.dram_tensor("wqm", [MO, C, 128], bf16, kind="ExternalInput")
    wkm = nc.dram_tensor("wkm", [MO, C, 128], bf16, kind="ExternalInput")
    wv = nc.dram_tensor("wv", [C, HD], bf16, kind="ExternalInput")
    wp = nc.dram_tensor("wp", [C, C], bf16, kind="ExternalInput")
    bp = nc.dram_tensor("bp", [1, C], bf16, kind="ExternalInput")
    mask = nc.dram_tensor("mask", [128, 128], bf16, kind="ExternalInput")
    out = nc.dram_tensor("out", [b_loc, T, C], f32, kind="ExternalOutput")

    with TileContext(nc) as tc:
        with (
            tc.tile_pool(name="weights", bufs=1) as wpool,
            tc.tile_pool(name="acts", bufs=2) as xpool,
            tc.tile_pool(name="attn", bufs=3) as apool,
            tc.tile_pool(name="small", bufs=4) as spool,
            tc.tile_pool(name="ons", bufs=2) as onpool,
            tc.tile_pool(name="outs", bufs=2) as opool,
            tc.tile_pool(name="psS", bufs=1, space="PSUM") as psA,
            tc.tile_pool(name="psAV", bufs=2, space="PSUM") as psB,
            tc.tile_pool(name="ps1", bufs=3, space="PSUM") as psC,
        ):
            # ---- persistent weights ----
            wq_sb = wpool.tile([128, KO, HD], bf16, name="wq_sb")
            wk_sb = wpool.tile([128, KO, HD], bf16, name="wk_sb")
            wv_sb = wpool.tile([128, KO, HD], bf16, name="wv_sb")
            wp_sb = wpool.tile([128, KO, C], bf16, name="wp_sb")
            # tiny inputs first: bias + mask must not land behind MBs of
            # weights (bias-broadcast matmuls sit early in the PE queue)
            bp1_sb = wpool.tile([1, C], bf16, name="bp1_sb")
            nc.sync.dma_start(out=bp1_sb, in_=bp[:])
            mask_sb = wpool.tile([128, 128], bf16, name="mask_sb")
            nc.sync.dma_start(out=mask_sb, in_=mask[:])
            # m-major: weights go on the SCALAR queue, batch-0 x chunks on the
            # sync queue — two DMA rings in parallel so the first Q chains are
            # not DMA-starved during the startup ramp
            xT0_sb = xpool.tile([128, KO, T], bf16, name="xT0_sb", tag="xT")
            nc.scalar.dma_start(
                out=wq_sb[:, :, ts(0, 128)],
                in_=wqm[0].rearrange("(ko p) f -> p ko f", p=128),
            )
            for k in range(KO):
                nc.sync.dma_start(out=xT0_sb[:, k, :], in_=xT[0, ds(128 * k, 128), :])
            for m in range(1, MO):
                nc.scalar.dma_start(
                    out=wq_sb[:, :, ts(m, 128)],
                    in_=wqm[m].rearrange("(ko p) f -> p ko f", p=128),
                )
            for m in range(MO):
                nc.scalar.dma_start(
                    out=wk_sb[:, :, ts(m, 128)],
                    in_=wkm[m].rearrange("(ko p) f -> p ko f", p=128),
                )
            nc.sync.dma_start(
                out=wv_sb, in_=wv[:].rearrange("(ko p) n -> p ko n", p=128)
            )
            nc.sync.dma_start(
                out=wp_sb, in_=wp[:].rearrange("(ko p) n -> p ko n", p=128)
            )
            # bias broadcast to all 128 partitions, f32, built once
            ones1_sb = wpool.tile([1, 128], bf16, name="ones1_sb")
            nc.gpsimd.memset(ones1_sb, 1.0)
            bias_bc = wpool.tile([128, C], f32, name="bias_bc")
            for half in range(2):
                psb = psC.tile([128, 512], f32, name="psb", tag="ps1")
                nc.tensor.matmul(
                    psb, ones1_sb, bp1_sb[:, ts(half, 512)], start=True, stop=True
                )
                nc.vector.tensor_copy(out=bias_bc[:, ts(half, 512)], in_=psb)
            # K^T in per-head zero-padded layout; two persistent slots for
            # cross-batch overlap. Zero halves are written once, ever.
            kT2_tiles = []
            for slot in range(2):
                t_ = wpool.tile([128, H, T], bf16, name=f"kT2_{slot}")
                nc.gpsimd.memset(t_, 0.0)
                kT2_tiles.append(t_)

            # ---------- per-batch tile state ----------
            state = {}

            def alloc_batch(b):
                if b == 0:
                    xT_sb = xT0_sb
                else:
                    xT_sb = xpool.tile([128, KO, T], bf16, name="xT_sb", tag="xT")
                qT_sb = xpool.tile([128, MO, T], bf16, name="qT_sb", tag="qT")
                v_sb = xpool.tile([128, TCH, H, 65], bf16, name="v_sb", tag="v")
                state[b] = dict(xT=xT_sb, qT=qT_sb, v=v_sb, kT2=kT2_tiles[b % 2])

            def issue_x_dma(b):
                xT_sb = state[b]["xT"]
                for k in range(KO):
                    nc.sync.dma_start(
                        out=xT_sb[:, k, :], in_=xT[b, ds(128 * k, 128), :]
                    )

            # ---------- QKV chain emitters (each: one 8-matmul psum chain) ----------
            def q_chain(b, m):
                st = state[b]
                ps = psC.tile([128, T], f32, name="ps_q", tag="ps1")
                for k in range(KO):
                    nc.tensor.matmul(
                        ps,
                        wq_sb[:, k, ts(m, 128)],
                        st["xT"][:, k, :],
                        start=(k == 0),
                        stop=(k == KO - 1),
                    )
                nc.scalar.copy(out=st["qT"][:, m, :], in_=ps)

            def k_chain(b, m):
                st = state[b]
                kT2 = st["kT2"]
                ps = psC.tile([128, T], f32, name="ps_k", tag="ps1")
                for k in range(KO):
                    nc.tensor.matmul(
                        ps,
                        wk_sb[:, k, ts(m, 128)],
                        st["xT"][:, k, :],
                        start=(k == 0),
                        stop=(k == KO - 1),
                    )
                # head 2m -> partitions 0:64, head 2m+1 -> partitions 64:128
                nc.vector.tensor_copy(out=kT2[0:64, 2 * m, :], in_=ps[0:64, :])
                nc.vector.tensor_copy(out=kT2[64:128, 2 * m + 1, :], in_=ps[64:128, :])

            def v_chain(b, i, half):
                st = state[b]
                v_sb = st["v"]
                if i == 0 and half == 0:
                    nc.vector.memset(v_sb[:, :, :, 64:65], 1.0)
                ps = psC.tile([128, 512], f32, name="ps_v", tag="ps1")
                for k in range(KO):
                    nc.tensor.matmul(
                        ps,
                        st["xT"][:, k, ts(i, 128)],
                        wv_sb[:, k, ts(half, 512)],
                        start=(k == 0),
                        stop=(k == KO - 1),
                    )
                nc.scalar.copy(
                    out=v_sb[:, i, 8 * half : 8 * half + 8, 0:64],
                    in_=ps.rearrange("p (h d) -> p h d", d=64),
                )

            def chain_pieces(b):
                """QKV chains of batch b grouped by deadline: K and the first
                Q pairs / half-0 V chunks are needed before batch b's first
                score matmul; Q m4-7 and half-1 V only from head 8 on."""
                kc = [lambda b=b, m=m: k_chain(b, m) for m in range(MO)]
                q03 = [lambda b=b, m=m: q_chain(b, m) for m in range(4)]
                q47 = [lambda b=b, m=m: q_chain(b, m) for m in range(4, MO)]
                v0 = [lambda b=b, i=i: v_chain(b, i, 0) for i in range(TCH)]
                v1 = [lambda b=b, i=i: v_chain(b, i, 1) for i in range(TCH)]
                return kc, q03, q47, v0, v1

            # ---------- attention pieces ----------
            def scores_head(b, h, aT):
                """scores^T blocks + exp for head h into aT (bf16)."""
                st = state[b]
                pair = h // 2
                psS = psA.tile([128, PACK], f32, name="psS", tag="psS")
                for j in range(TCH):
                    nc.tensor.matmul(
                        psS[:, ds(off[j], widths[j])],
                        st["kT2"][:, h, ts(j, 128)],
                        st["qT"][:, pair, ds(128 * j, widths[j])],
                        start=True,
                        stop=True,
                    )
                nc.scalar.activation(aT, psS, AF.Exp, scale=SCALE)
                # zero the masked (s>t) part of the diagonal blocks (gpsimd)
                for j in range(TCH):
                    nc.gpsimd.tensor_mul(
                        aT[:, ds(off[j], 128)], aT[:, ds(off[j], 128)], mask_sb
                    )

            def av_head(b, h, aT, on_tiles):
                """attn @ [V | 1] for head h: four t-chunk groups accumulate
                into ONE psum bank [128, 4, 128]; merged reciprocal, then
                per-chunk per-partition scale into the on tiles."""
                st = state[b]
                pair, pb = h // 2, 64 * (h % 2)
                psAV = psB.tile([128, TCH, 128], f32, name="psAV", tag="psAV")
                for i in range(TCH):
                    for j in range(i + 1):
                        nc.tensor.matmul(
                            psAV[:, i, 0:65],
                            aT[:, ds(off[j] + 128 * (i - j), 128)],
                            st["v"][:, j, h, :],
                            start=(j == 0),
                            stop=(j == i),
                        )
                rr = spool.tile([128, TCH], f32, name="rr", tag="rr")
                nc.vector.reciprocal(rr, psAV[:, :, 64:65])
                for i in range(TCH):
                    nc.vector.tensor_scalar_mul(
                        on_tiles[i][:, ds(128 * pair + pb, 64)],
                        psAV[:, i, 0:64],
                        rr[:, i : i + 1],
                    )

            # ---------- batch 0 QKV (serial; DMA-order matched) ----------
            alloc_batch(0)
            for m in range(MO):
                q_chain(0, m)
            for m in range(MO):
                k_chain(0, m)
            for i in range(TCH):
                v_chain(0, i, 0)
            for i in range(TCH):
                v_chain(0, i, 1)

            # ---------- main loop: attention(b) fused with QKV(b+1) ----------
            held_back = {}
            for b in range(b_loc):
                nxt = b + 1 if b + 1 < b_loc else None
                fillers = []
                if nxt is not None:
                    alloc_batch(nxt)
                    issue_x_dma(nxt)
                    kc, q03, q47, v0, v1 = chain_pieces(nxt)
                    if nxt == b_loc - 1:
                        # last batch keeps its late chains as SELF-fillers for
                        # its own attention (nothing else hides exp latency)
                        fillers = kc + q03 + v0
                        held_back[nxt] = v1 + q47
                    else:
                        fillers = kc + q03 + v0 + q47 + v1
                fi = 0  # next filler index
                # self-fillers: V(i,1) consumed from head 8 (step 9), Q m4-7
                # from scores(8/10/12/14) — slots h=2..9 meet every deadline
                self_fill = held_back.pop(b, [])

                on_tiles = [
                    onpool.tile([128, HD], bf16, name=f"on{i}", tag=f"on{i}")
                    for i in range(TCH)
                ]
                aT_tiles = {}
                for h in range(H):
                    aT = apool.tile([128, PACK], bf16, name="aT", tag="aT")
                    aT_tiles[h] = aT
                    scores_head(b, h, aT)
                    if h >= 1:
                        av_head(b, h - 1, aT_tiles.pop(h - 1), on_tiles)
                    if self_fill and 2 <= h <= 9:
                        self_fill[h - 2]()
                    elif h >= 2 and fi < len(fillers):
                        fillers[fi]()
                        fi += 1
                av_head(b, H - 1, aT_tiles.pop(H - 1), on_tiles)
                for _ in range(2):
                    if fi < len(fillers):
                        fillers[fi]()
                        fi += 1

                # batched head-concat transpose: [t, hd] -> [hd, t] per t-chunk
                outT_sb = opool.tile([128, MO, T], bf16, name="outT_sb", tag="outT")
                for i in range(TCH):
                    nc.sync.dma_start_transpose(
                        out=outT_sb[:, :, ts(i, 128)], in_=on_tiles[i]
                    )

                # ---- final projection; bias added during PSUM evacuation ----
                for i in range(TCH):
                    out_sb = opool.tile([128, C], f32, name="out_sb", tag="out_sb")
                    for half in range(2):
                        psF = psC.tile([128, 512], f32, name="psF", tag="ps1")
                        for k in range(MO):
                            nc.tensor.matmul(
                                psF,
                                outT_sb[:, k, ts(i, 128)],
                                wp_sb[:, k, ts(half, 512)],
                                start=(k == 0),
                                stop=(k == MO - 1),
                            )
                        nc.vector.tensor_add(
                            out=out_sb[:, ts(half, 512)],
                            in0=psF,
                            in1=bias_bc[:, ts(half, 512)],
                        )
                        if fi < len(fillers):
                            fillers[fi]()
                            fi += 1
                    nc.sync.dma_start(out=out[b, ts(i, 128), :], in_=out_sb)
                # any leftover fillers (shouldn't happen: 14+2+8 = 24)
                while fi < len(fillers):
                    fillers[fi]()
                    fi += 1

    nc.compile()
    return nc


def make_in_maps(x, wq, wk, wv, w_proj, b_proj, b_loc=B_LOC, ncores=NCORES):
    bf16 = ml_dtypes.bfloat16
    MO = HD // 128
    x = np.asarray(x, dtype=np.float32)
    # host-side layout prep (transpose / reshape / cast only)
    xT = np.ascontiguousarray(x.transpose(0, 2, 1)).astype(bf16)  # [B, C, T]
    wq2 = np.asarray(wq, np.float32).transpose(1, 0, 2).reshape(C, HD)
    wk2 = np.asarray(wk, np.float32).transpose(1, 0, 2).reshape(C, HD)
    # m-major: [MO, C, 128]
    wqm = np.ascontiguousarray(wq2.reshape(C, MO, 128).transpose(1, 0, 2)).astype(bf16)
    wkm = np.ascontiguousarray(wk2.reshape(C, MO, 128).transpose(1, 0, 2)).astype(bf16)
    wv2 = np.ascontiguousarray(
        np.asarray(wv, np.float32).transpose(1, 0, 2).reshape(C, HD)
    ).astype(bf16)
    wp2 = np.ascontiguousarray(np.asarray(w_proj, np.float32)).astype(bf16)
    bp2 = np.asarray(b_proj, np.float32).reshape(1, C).astype(bf16)
    # mask[p, f] = 1 where p <= f (valid: s_in <= t_in on diagonal blocks)
    m = np.triu(np.ones((128, 128), np.float32)).astype(bf16)
    in_maps = []
    for c in range(ncores):
        in_maps.append(
            {
                "xT": xT[c * b_loc : (c + 1) * b_loc],
                "wqm": wqm,
                "wkm": wkm,
                "wv": wv2,
                "wp": wp2,
                "bp": bp2,
                "mask": m,
            }
        )
    return in_maps


def kernel(x, wq, wk, wv, w_proj, b_proj, **run_kwargs):
    from concourse import bass_utils

    if "nc" not in _CACHE:
        _CACHE["nc"] = build_nc(B_LOC)
    nc = _CACHE["nc"]
    in_maps = make_in_maps(x, wq, wk, wv, w_proj, b_proj)
    res = bass_utils.run_bass_kernel_spmd(
        nc, in_maps, core_ids=list(range(NCORES)), **run_kwargs
    )
    outs = [r["out"] for r in res.results]
    full = np.concatenate(outs, axis=0).astype(np.float32)
    if run_kwargs:
        _CACHE["last_result"] = res
    return full


# revision 8
# speedup vs baseline: 1.0850x; 1.0850x over previous
"""Causal multi-head attention (B=32,T=512,C=1024,H=16,D=64) on 8 TRN2 cores.

Strategy: pure data-parallel over the batch axis (4 batches per core, no
collectives). Per core, per batch:
  - x^T [C,T] arrives pre-transposed from the host (layout prep only).
  - Q^T [HD,T] and V [T,HD] computed with bf16 matmuls (fp32 PSUM); K^T is
    written into per-head zero-padded [128,T] tiles so every PE matmul runs
    in the full 128x128 array mode (no tiling-mode switches/drains).
  - scores^T [s,t] blocks computed directly on PE (only the causal lower
    triangle of [T,T], packed into a [128,1280] PSUM tile per head).
  - softmax without max-subtraction: scores here are bounded (|s|<~3) so
    exp is safe in fp32; masked entries are zeroed by multiplying the
    exp'd diagonal blocks with a 0/1 triangular mask (gpsimd).
  - attn@V with a ones-augmented V column producing the softmax row-sums
    in the same matmul; all four t-chunk AV groups accumulate into ONE
    psum bank [128,4,128]; merged reciprocal + per-partition scales.
  - head-concat transpose via one batched DMA-transpose per t-chunk
    (issued from the sync queue); final projection with bias added during
    PSUM evacuation; fp32 output.

Pipelining (v2): the attention phase of batch b is engine-cadence bound
(exp on scalar ~1.33us/head, masks on gpsimd ~1.7us/head) while the PE
only has ~1.1us/head of matmul work.  To keep the PE saturated, batch
b+1's 24 QKV psum-chains are emitted as FILLER between head iterations
of batch b's attention and between proj chains, so the in-order PE queue
always has independent work while exp/mask/normalize latencies resolve.
Weights are DMA'd in m-major order so the first Q chain of batch 0 can
start after ~400KB instead of ~8MB.
"""

import sys

if "/opt/trn_rl_repo" not in sys.path:
    sys.path.insert(0, "/opt/trn_rl_repo")

import numpy as np
import ml_dtypes

B, T, C = 32, 512, 1024
H, D = 16, 64
HD = H * D
NCORES = 8
B_LOC = B // NCORES

_CACHE = {}


def build_nc(b_loc=B_LOC):
    import concourse.mybir as mybir
    from concourse import bacc
    from concourse.bass import ds, ts
    from concourse.tile import TileContext

    f32 = mybir.dt.float32
    bf16 = mybir.dt.bfloat16
    AF = mybir.ActivationFunctionType

    KO = C // 128  # 8 contraction chunks
    MO = HD // 128  # 8 output-row chunks
    TCH = T // 128  # 4 t-chunks
    SCALE = 1.0 / float(np.sqrt(C))

    # scores^T causal packing: s-chunk j covers t in [128j, T), width T-128j.
    # Packed into one PSUM tile [128, 1280] so no matmul output crosses a
    # 2KB bank boundary: j0@[0,512) bank0, j1@[512,896) bank1,
    # j3@[896,1024) bank1, j2@[1024,1280) bank2.
    widths = [T - 128 * j for j in range(TCH)]
    off = [0, 512, 1024, 896]
    PACK = 1280

    nc = bacc.Bacc("TRN2", target_bir_lowering=False)
    xT = nc.dram_tensor("xT", [b_loc, C, T], bf16, kind="ExternalInput")
    # m-major weight layouts: [MO, C, 128] so column-chunk m is contiguous
    wqm = nc.dram_tensor("wqm", [MO, C, 128], bf16, kind="ExternalInput")
    wkm = nc.dram_tensor("wkm", [MO, C, 128], bf16, kind="ExternalInput")
    wv = nc.dram_tensor("wv", [C, HD], bf16, kind="ExternalInput")
    wp = nc.dram_tensor("wp", [C, C], bf16, kind="ExternalInput")
    bp = nc.dram_tensor("bp", [1, C], bf16, kind="ExternalInput")
    mask = nc.dram_tensor("mask", [128, 128], bf16, kind="ExternalInput")
    out = nc.dram_tensor("out", [b_loc, T, C], f32, kind="ExternalOutput")

    with TileContext(nc) as tc:
        with (
            tc.tile_pool(name="weights", bufs=1) as wpool,
            tc.tile_pool(name="acts", bufs=2) as xpool,
            tc.tile_pool(name="attn", bufs=3) as apool,
            tc.tile_pool(name="small", bufs=4) as spool,
            tc.tile_pool(name="ons", bufs=2) as onpool,
            tc.tile_pool(name="outs", bufs=2) as opool,
            tc.tile_pool(name="psS", bufs=1, space="PSUM") as psA,
            tc.tile_pool(name="psAV", bufs=2, space="PSUM") as psB,
            tc.tile_pool(name="ps1", bufs=3, space="PSUM") as psC,
        ):
            # ---- persistent weights ----
            wq_sb = wpool.tile([128, KO, HD], bf16, name="wq_sb")
            wk_sb = wpool.tile([128, KO, HD], bf16, name="wk_sb")
            wv_sb = wpool.tile([128, KO, HD], bf16, name="wv_sb")
            wp_sb = wpool.tile([128, KO, C], bf16, name="wp_sb")
            # tiny inputs first: bias + mask must not land behind MBs of
            # weights (bias-broadcast matmuls sit early in the PE queue)
            bp1_sb = wpool.tile([1, C], bf16, name="bp1_sb")
            nc.sync.dma_start(out=bp1_sb, in_=bp[:])
            mask_sb = wpool.tile([128, 128], bf16, name="mask_sb")
            nc.sync.dma_start(out=mask_sb, in_=mask[:])
            # m-major: weights go on the SCALAR queue, batch-0 x chunks on the
            # sync queue — two DMA rings in parallel so the first Q chains are
            # not DMA-starved during the startup ramp
            xT0_sb = xpool.tile([128, KO, T], bf16, name="xT0_sb", tag="xT")
            for k in range(KO):
                nc.scalar.dma_start(out=xT0_sb[:, k, :], in_=xT[0, ds(128 * k, 128), :])
            for m in range(MO):
                nc.sync.dma_start(
                    out=wq_sb[:, :, ts(m, 128)],
                    in_=wqm[m].rearrange("(ko p) f -> p ko f", p=128),
                )
            for m in range(MO):
                nc.sync.dma_start(
                    out=wk_sb[:, :, ts(m, 128)],
                    in_=wkm[m].rearrange("(ko p) f -> p ko f", p=128),
                )
            nc.sync.dma_start(
                out=wv_sb, in_=wv[:].rearrange("(ko p) n -> p ko n", p=128)
            )
            nc.sync.dma_start(
                out=wp_sb, in_=wp[:].rearrange("(ko p) n -> p ko n", p=128)
            )
            # bias broadcast to all 128 partitions, f32, built once
            ones1_sb = wpool.tile([1, 128], bf16, name="ones1_sb")
            nc.gpsimd.memset(ones1_sb, 1.0)
            bias_bc = wpool.tile([128, C], f32, name="bias_bc")
            for half in range(2):
                psb = psC.tile([128, 512], f32, name="psb", tag="ps1")
                nc.tensor.matmul(
                    psb, ones1_sb, bp1_sb[:, ts(half, 512)], start=True, stop=True
                )
                nc.vector.tensor_copy(out=bias_bc[:, ts(half, 512)], in_=psb)
            # K^T in per-head zero-padded layout; two persistent slots for
            # cross-batch overlap. Zero halves are written once, ever.
            kT2_tiles = []
            for slot in range(2):
                t_ = wpool.tile([128, H, T], bf16, name=f"kT2_{slot}")
                nc.gpsimd.memset(t_, 0.0)
                kT2_tiles.append(t_)

            # ---------- per-batch tile state ----------
            state = {}

            def alloc_batch(b):
                if b == 0:
                    xT_sb = xT0_sb
                else:
                    xT_sb = xpool.tile([128, KO, T], bf16, name="xT_sb", tag="xT")
                qT_sb = xpool.tile([128, MO, T], bf16, name="qT_sb", tag="qT")
                v_sb = xpool.tile([128, TCH, H, 65], bf16, name="v_sb", tag="v")
                state[b] = dict(xT=xT_sb, qT=qT_sb, v=v_sb, kT2=kT2_tiles[b % 2])

            def issue_x_dma(b):
                xT_sb = state[b]["xT"]
                for k in range(KO):
                    nc.sync.dma_start(
                        out=xT_sb[:, k, :], in_=xT[b, ds(128 * k, 128), :]
                    )

            # ---------- QKV chain emitters (each: one 8-matmul psum chain) ----------
            def q_chain(b, m):
                st = state[b]
                ps = psC.tile([128, T], f32, name="ps_q", tag="ps1")
                for k in range(KO):
                    nc.tensor.matmul(
                        ps,
                        wq_sb[:, k, ts(m, 128)],
                        st["xT"][:, k, :],
                        start=(k == 0),
                        stop=(k == KO - 1),
                    )
                nc.scalar.copy(out=st["qT"][:, m, :], in_=ps)

            def k_chain(b, m):
                st = state[b]
                kT2 = st["kT2"]
                ps = psC.tile([128, T], f32, name="ps_k", tag="ps1")
                for k in range(KO):
                    nc.tensor.matmul(
                        ps,
                        wk_sb[:, k, ts(m, 128)],
                        st["xT"][:, k, :],
                        start=(k == 0),
                        stop=(k == KO - 1),
                    )
                # head 2m -> partitions 0:64, head 2m+1 -> partitions 64:128
                nc.vector.tensor_copy(out=kT2[0:64, 2 * m, :], in_=ps[0:64, :])
                nc.vector.tensor_copy(out=kT2[64:128, 2 * m + 1, :], in_=ps[64:128, :])

            def v_chain(b, i, half):
                st = state[b]
                v_sb = st["v"]
                if i == 0 and half == 0:
                    nc.vector.memset(v_sb[:, :, :, 64:65], 1.0)
                ps = psC.tile([128, 512], f32, name="ps_v", tag="ps1")
                for k in range(KO):
                    nc.tensor.matmul(
                        ps,
                        st["xT"][:, k, ts(i, 128)],
                        wv_sb[:, k, ts(half, 512)],
                        start=(k == 0),
                        stop=(k == KO - 1),
                    )
                nc.scalar.copy(
                    out=v_sb[:, i, 8 * half : 8 * half + 8, 0:64],
                    in_=ps.rearrange("p (h d) -> p h d", d=64),
                )

            def chain_pieces(b):
                """QKV chains of batch b grouped by deadline: K and the first
                Q pairs / half-0 V chunks are needed before batch b's first
                score matmul; Q m4-7 and half-1 V only from head 8 on."""
                kc = [lambda b=b, m=m: k_chain(b, m) for m in range(MO)]
                q03 = [lambda b=b, m=m: q_chain(b, m) for m in range(4)]
                q47 = [lambda b=b, m=m: q_chain(b, m) for m in range(4, MO)]
                v0 = [lambda b=b, i=i: v_chain(b, i, 0) for i in range(TCH)]
                v1 = [lambda b=b, i=i: v_chain(b, i, 1) for i in range(TCH)]
                return kc, q03, q47, v0, v1

            # ---------- attention pieces ----------
            def scores_head(b, h, aT):
                """scores^T blocks + exp for head h into aT (bf16)."""
                st = state[b]
                pair = h // 2
                psS = psA.tile([128, PACK], f32, name="psS", tag="psS")
                for j in range(TCH):
                    nc.tensor.matmul(
                        psS[:, ds(off[j], widths[j])],
                        st["kT2"][:, h, ts(j, 128)],
                        st["qT"][:, pair, ds(128 * j, widths[j])],
                        start=True,
                        stop=True,
                    )
                nc.scalar.activation(aT, psS, AF.Exp, scale=SCALE)
                # zero the masked (s>t) part of the diagonal blocks (gpsimd)
                for j in range(TCH):
                    nc.gpsimd.tensor_mul(
                        aT[:, ds(off[j], 128)], aT[:, ds(off[j], 128)], mask_sb
                    )

            def av_head(b, h, aT, on_tiles):
                """attn @ [V | 1] for head h: four t-chunk groups accumulate
                into ONE psum bank [128, 4, 128]; merged reciprocal, then
                per-chunk per-partition scale into the on tiles."""
                st = state[b]
                pair, pb = h // 2, 64 * (h % 2)
                psAV = psB.tile([128, TCH, 128], f32, name="psAV", tag="psAV")
                for i in range(TCH):
                    for j in range(i + 1):
                        nc.tensor.matmul(
                            psAV[:, i, 0:65],
                            aT[:, ds(off[j] + 128 * (i - j), 128)],
                            st["v"][:, j, h, :],
                            start=(j == 0),
                            stop=(j == i),
                        )
                rr = spool.tile([128, TCH], f32, name="rr", tag="rr")
                nc.vector.reciprocal(rr, psAV[:, :, 64:65])
                for i in range(TCH):
                    nc.vector.tensor_scalar_mul(
                        on_tiles[i][:, ds(128 * pair + pb, 64)],
                        psAV[:, i, 0:64],
                        rr[:, i : i + 1],
                    )

            # ---------- batch 0 QKV (serial; DMA-order matched) ----------
            alloc_batch(0)
            for m in range(MO):
                q_chain(0, m)
            for m in range(MO):
                k_chain(0, m)
            for i in range(TCH):
                v_chain(0, i, 0)
            for i in range(TCH):
                v_chain(0, i, 1)

            # ---------- main loop: attention(b) fused with QKV(b+1) ----------
            held_back = {}
            for b in range(b_loc):
                nxt = b + 1 if b + 1 < b_loc else None
                fillers = []
                if nxt is not None:
                    alloc_batch(nxt)
                    issue_x_dma(nxt)
                    kc, q03, q47, v0, v1 = chain_pieces(nxt)
                    if nxt == b_loc - 1:
                        # last batch keeps its late chains as SELF-fillers for
                        # its own attention (nothing else hides exp latency)
                        fillers = kc + q03 + v0
                        held_back[nxt] = v1 + q47
                    else:
                        fillers = kc + q03 + v0 + q47 + v1
                fi = 0  # next filler index
                # self-fillers: V(i,1) consumed from head 8 (step 9), Q m4-7
                # from scores(8/10/12/14) — slots h=2..9 meet every deadline
                self_fill = held_back.pop(b, [])

                on_tiles = [
                    onpool.tile([128, HD], bf16, name=f"on{i}", tag=f"on{i}")
                    for i in range(TCH)
                ]
                aT_tiles = {}
                for h in range(H):
                    aT = apool.tile([128, PACK], bf16, name="aT", tag="aT")
                    aT_tiles[h] = aT
                    scores_head(b, h, aT)
                    if h >= 1:
                        av_head(b, h - 1, aT_tiles.pop(h - 1), on_tiles)
                    if self_fill and 2 <= h <= 9:
                        self_fill[h - 2]()
                    elif h >= 2 and fi < len(fillers):
                        fillers[fi]()
                        fi += 1
                av_head(b, H - 1, aT_tiles.pop(H - 1), on_tiles)
                for _ in range(2):
                    if fi < len(fillers):
                        fillers[fi]()
                        fi += 1

                # batched head-concat transpose: [t, hd] -> [hd, t] per t-chunk
                outT_sb = opool.tile([128, MO, T], bf16, name="outT_sb", tag="outT")
                for i in range(TCH):
                    nc.sync.dma_start_transpose(
                        out=outT_sb[:, :, ts(i, 128)], in_=on_tiles[i]
                    )

                # ---- final projection; bias added during PSUM evacuation ----
                for i in range(TCH):
                    out_sb = opool.tile([128, C], f32, name="out_sb", tag="out_sb")
                    for half in range(2):
                        psF = psC.tile([128, 512], f32, name="psF", tag="ps1")
                        for k in range(MO):
                            nc.tensor.matmul(
                                psF,
                                outT_sb[:, k, ts(i, 128)],
                                wp_sb[:, k, ts(half, 512)],
                                start=(k == 0),
                                stop=(k == MO - 1),
                            )
                        nc.vector.tensor_add(
                            out=out_sb[:, ts(half, 512)],
                            in0=psF,
                            in1=bias_bc[:, ts(half, 512)],
                        )
                        if fi < len(fillers):
                            fillers[fi]()
                            fi += 1
                    nc.sync.dma_start(out=out[b, ts(i, 128), :], in_=out_sb)
                # any leftover fillers (shouldn't happen: 14+2+8 = 24)
                while fi < len(fillers):
                    fillers[fi]()
                    fi += 1

    nc.compile()
    return nc


def make_in_maps(x, wq, wk, wv, w_proj, b_proj, b_loc=B_LOC, ncores=NCORES):
    bf16 = ml_dtypes.bfloat16
    MO = HD // 128
    x = np.asarray(x, dtype=np.float32)
    # host-side layout prep (transpose / reshape / cast only)
    xT = np.ascontiguousarray(x.transpose(0, 2, 1)).astype(bf16)  # [B, C, T]
    wq2 = np.asarray(wq, np.float32).transpose(1, 0, 2).reshape(C, HD)
    wk2 = np.asarray(wk, np.float32).transpose(1, 0, 2).reshape(C, HD)
    # m-major: [MO, C, 128]
    wqm = np.ascontiguousarray(wq2.reshape(C, MO, 128).transpose(1, 0, 2)).astype(bf16)
    wkm = np.ascontiguousarray(wk2.reshape(C, MO, 128).transpose(1, 0, 2)).astype(bf16)
    wv2 = np.ascontiguousarray(
        np.asarray(wv, np.float32).transpose(1, 0, 2).reshape(C, HD)
    ).astype(bf16)
    wp2 = np.ascontiguousarray(np.asarray(w_proj, np.float32)).astype(bf16)
    bp2 = np.asarray(b_proj, np.float32).reshape(1, C).astype(bf16)
    # mask[p, f] = 1 where p <= f (valid: s_in <= t_in on diagonal blocks)
    m = np.triu(np.ones((128, 128), np.float32)).astype(bf16)
    in_maps = []
    for c in range(ncores):
        in_maps.append(
            {
                "xT": xT[c * b_loc : (c + 1) * b_loc],
                "wqm": wqm,
                "wkm": wkm,
                "wv": wv2,
                "wp": wp2,
                "bp": bp2,
                "mask": m,
            }
        )
    return in_maps


def kernel(x, wq, wk, wv, w_proj, b_proj, **run_kwargs):
    from concourse import bass_utils

    if "nc" not in _CACHE:
        _CACHE["nc"] = build_nc(B_LOC)
    nc = _CACHE["nc"]
    in_maps = make_in_maps(x, wq, wk, wv, w_proj, b_proj)
    res = bass_utils.run_bass_kernel_spmd(
        nc, in_maps, core_ids=list(range(NCORES)), **run_kwargs
    )
    outs = [r["out"] for r in res.results]
    full = np.concatenate(outs, axis=0).astype(np.float32)
    if run_kwargs:
        _CACHE["last_result"] = res
    return full


# revision 10
# speedup vs baseline: 1.0955x; 1.0097x over previous
"""Causal multi-head attention (B=32,T=512,C=1024,H=16,D=64) on 8 TRN2 cores.

Strategy: pure data-parallel over the batch axis (4 batches per core, no
collectives). Per core, per batch:
  - x^T [C,T] arrives pre-transposed from the host (layout prep only).
  - Q^T [HD,T] and V [T,HD] computed with bf16 matmuls (fp32 PSUM); K^T is
    written into per-head zero-padded [128,T] tiles so every PE matmul runs
    in the full 128x128 array mode (no tiling-mode switches/drains).
  - scores^T [s,t] blocks computed directly on PE (only the causal lower
    triangle of [T,T], packed into a [128,1280] PSUM tile per head).
  - softmax without max-subtraction: scores here are bounded (|s|<~3) so
    exp is safe in fp32; masked entries are zeroed by multiplying the
    exp'd diagonal blocks with a 0/1 triangular mask (gpsimd).
  - attn@V with a ones-augmented V column producing the softmax row-sums
    in the same matmul; all four t-chunk AV groups accumulate into ONE
    psum bank [128,4,128]; merged reciprocal + per-partition scales.
  - head-concat transpose via one batched DMA-transpose per t-chunk
    (issued from the sync queue); final projection with bias added during
    PSUM evacuation; fp32 output.

Pipelining (v2): the attention phase of batch b is engine-cadence bound
(exp on scalar ~1.33us/head, masks on gpsimd ~1.7us/head) while the PE
only has ~1.1us/head of matmul work.  To keep the PE saturated, batch
b+1's 24 QKV psum-chains are emitted as FILLER between head iterations
of batch b's attention and between proj chains, so the in-order PE queue
always has independent work while exp/mask/normalize latencies resolve.
Weights are DMA'd in m-major order so the first Q chain of batch 0 can
start after ~400KB instead of ~8MB.
"""

import sys

if "/opt/trn_rl_repo" not in sys.path:
    sys.path.insert(0, "/opt/trn_rl_repo")

import numpy as np
import ml_dtypes

B, T, C = 32, 512, 1024
H, D = 16, 64
HD = H * D
NCORES = 8
B_LOC = B // NCORES

_CACHE = {}


def build_nc(b_loc=B_LOC):
    import concourse.mybir as mybir
    from concourse import bacc
    from concourse.bass import ds, ts
    from concourse.tile import TileContext

    f32 = mybir.dt.float32
    bf16 = mybir.dt.bfloat16
    AF = mybir.ActivationFunctionType

    KO = C // 128  # 8 contraction chunks
    MO = HD // 128  # 8 output-row chunks
    TCH = T // 128  # 4 t-chunks
    SCALE = 1.0 / float(np.sqrt(C))

    # scores^T causal packing: s-chunk j covers t in [128j, T), width T-128j.
    # Packed into one PSUM tile [128, 1280] so no matmul output crosses a
    # 2KB bank boundary: j0@[0,512) bank0, j1@[512,896) bank1,
    # j3@[896,1024) bank1, j2@[1024,1280) bank2.
    widths = [T - 128 * j for j in range(TCH)]
    off = [0, 512, 1024, 896]
    PACK = 1280

    nc = bacc.Bacc("TRN2", target_bir_lowering=False)
    xT = nc.dram_tensor("xT", [b_loc, C, T], bf16, kind="ExternalInput")
    # m-major weight layouts: [MO, C, 128] so column-chunk m is contiguous
    wqm = nc.dram_tensor("wqm", [MO, C, 128], bf16, kind="ExternalInput")
    wkm = nc.dram_tensor("wkm", [MO, C, 128], bf16, kind="ExternalInput")
    wv = nc.dram_tensor("wv", [C, HD], bf16, kind="ExternalInput")
    wp = nc.dram_tensor("wp", [C, C], bf16, kind="ExternalInput")
    bp = nc.dram_tensor("bp", [1, C], bf16, kind="ExternalInput")
    mask = nc.dram_tensor("mask", [128, 128], bf16, kind="ExternalInput")
    out = nc.dram_tensor("out", [b_loc, T, C], f32, kind="ExternalOutput")

    with TileContext(nc) as tc:
        with (
            tc.tile_pool(name="weights", bufs=1) as wpool,
            tc.tile_pool(name="acts", bufs=2) as xpool,
            tc.tile_pool(name="attn", bufs=3) as apool,
            tc.tile_pool(name="small", bufs=4) as spool,
            tc.tile_pool(name="ons", bufs=2) as onpool,
            tc.tile_pool(name="outs", bufs=2) as opool,
            tc.tile_pool(name="psS", bufs=1, space="PSUM") as psA,
            tc.tile_pool(name="psAV", bufs=2, space="PSUM") as psB,
            tc.tile_pool(name="ps1", bufs=3, space="PSUM") as psC,
        ):
            # ---- persistent weights ----
            wq_sb = wpool.tile([128, KO, HD], bf16, name="wq_sb")
            wk_sb = wpool.tile([128, KO, HD], bf16, name="wk_sb")
            wv_sb = wpool.tile([128, KO, HD], bf16, name="wv_sb")
            wp_sb = wpool.tile([128, KO, C], bf16, name="wp_sb")
            # tiny inputs first: bias + mask must not land behind MBs of
            # weights (bias-broadcast matmuls sit early in the PE queue)
            bp1_sb = wpool.tile([1, C], bf16, name="bp1_sb")
            nc.sync.dma_start(out=bp1_sb, in_=bp[:])
            mask_sb = wpool.tile([128, 128], bf16, name="mask_sb")
            nc.sync.dma_start(out=mask_sb, in_=mask[:])
            # m-major: weights go on the SCALAR queue, batch-0 x chunks on the
            # sync queue — two DMA rings in parallel so the first Q chains are
            # not DMA-starved during the startup ramp
            xT0_sb = xpool.tile([128, KO, T], bf16, name="xT0_sb", tag="xT")
            for k in range(KO):
                eng = nc.scalar if k % 2 == 0 else nc.gpsimd
                eng.dma_start(out=xT0_sb[:, k, :], in_=xT[0, ds(128 * k, 128), :])
            for m in range(MO):
                nc.sync.dma_start(
                    out=wq_sb[:, :, ts(m, 128)],
                    in_=wqm[m].rearrange("(ko p) f -> p ko f", p=128),
                )
            for m in range(MO):
                nc.sync.dma_start(
                    out=wk_sb[:, :, ts(m, 128)],
                    in_=wkm[m].rearrange("(ko p) f -> p ko f", p=128),
                )
            nc.sync.dma_start(
                out=wv_sb, in_=wv[:].rearrange("(ko p) n -> p ko n", p=128)
            )
            nc.sync.dma_start(
                out=wp_sb, in_=wp[:].rearrange("(ko p) n -> p ko n", p=128)
            )
            # bias broadcast to all 128 partitions, f32, built once
            ones1_sb = wpool.tile([1, 128], bf16, name="ones1_sb")
            nc.gpsimd.memset(ones1_sb, 1.0)
            bias_bc = wpool.tile([128, C], f32, name="bias_bc")
            for half in range(2):
                psb = psC.tile([128, 512], f32, name="psb", tag="ps1")
                nc.tensor.matmul(
                    psb, ones1_sb, bp1_sb[:, ts(half, 512)], start=True, stop=True
                )
                nc.vector.tensor_copy(out=bias_bc[:, ts(half, 512)], in_=psb)
            # K^T in per-head zero-padded layout; two persistent slots for
            # cross-batch overlap. Zero halves are written once, ever.
            kT2_tiles = []
            for slot in range(2):
                t_ = wpool.tile([128, H, T], bf16, name=f"kT2_{slot}")
                nc.gpsimd.memset(t_, 0.0)
                kT2_tiles.append(t_)

            # ---------- per-batch tile state ----------
            state = {}

            def alloc_batch(b):
                if b == 0:
                    xT_sb = xT0_sb
                else:
                    xT_sb = xpool.tile([128, KO, T], bf16, name="xT_sb", tag="xT")
                qT_sb = xpool.tile([128, MO, T], bf16, name="qT_sb", tag="qT")
                v_sb = xpool.tile([128, TCH, H, 65], bf16, name="v_sb", tag="v")
                state[b] = dict(xT=xT_sb, qT=qT_sb, v=v_sb, kT2=kT2_tiles[b % 2])

            def issue_x_dma(b):
                xT_sb = state[b]["xT"]
                for k in range(KO):
                    nc.sync.dma_start(
                        out=xT_sb[:, k, :], in_=xT[b, ds(128 * k, 128), :]
                    )

            # ---------- QKV chain emitters (each: one 8-matmul psum chain) ----------
            def q_chain(b, m):
                st = state[b]
                ps = psC.tile([128, T], f32, name="ps_q", tag="ps1")
                for k in range(KO):
                    nc.tensor.matmul(
                        ps,
                        wq_sb[:, k, ts(m, 128)],
                        st["xT"][:, k, :],
                        start=(k == 0),
                        stop=(k == KO - 1),
                    )
                nc.scalar.copy(out=st["qT"][:, m, :], in_=ps)

            def k_chain(b, m):
                st = state[b]
                kT2 = st["kT2"]
                ps = psC.tile([128, T], f32, name="ps_k", tag="ps1")
                for k in range(KO):
                    nc.tensor.matmul(
                        ps,
                        wk_sb[:, k, ts(m, 128)],
                        st["xT"][:, k, :],
                        start=(k == 0),
                        stop=(k == KO - 1),
                    )
                # head 2m -> partitions 0:64, head 2m+1 -> partitions 64:128
                nc.vector.tensor_copy(out=kT2[0:64, 2 * m, :], in_=ps[0:64, :])
                nc.vector.tensor_copy(out=kT2[64:128, 2 * m + 1, :], in_=ps[64:128, :])

            def v_chain(b, i, half):
                st = state[b]
                v_sb = st["v"]
                if i == 0 and half == 0:
                    nc.vector.memset(v_sb[:, :, :, 64:65], 1.0)
                ps = psC.tile([128, 512], f32, name="ps_v", tag="ps1")
                for k in range(KO):
                    nc.tensor.matmul(
                        ps,
                        st["xT"][:, k, ts(i, 128)],
                        wv_sb[:, k, ts(half, 512)],
                        start=(k == 0),
                        stop=(k == KO - 1),
                    )
                nc.scalar.copy(
                    out=v_sb[:, i, 8 * half : 8 * half + 8, 0:64],
                    in_=ps.rearrange("p (h d) -> p h d", d=64),
                )

            def chain_pieces(b):
                """QKV chains of batch b grouped by deadline: K and the first
                Q pairs / half-0 V chunks are needed before batch b's first
                score matmul; Q m4-7 and half-1 V only from head 8 on."""
                kc = [lambda b=b, m=m: k_chain(b, m) for m in range(MO)]
                q03 = [lambda b=b, m=m: q_chain(b, m) for m in range(4)]
                q47 = [lambda b=b, m=m: q_chain(b, m) for m in range(4, MO)]
                v0 = [lambda b=b, i=i: v_chain(b, i, 0) for i in range(TCH)]
                v1 = [lambda b=b, i=i: v_chain(b, i, 1) for i in range(TCH)]
                return kc, q03, q47, v0, v1

            # ---------- attention pieces ----------
            def scores_head(b, h, aT):
                """scores^T blocks + exp for head h into aT (bf16)."""
                st = state[b]
                pair = h // 2
                psS = psA.tile([128, PACK], f32, name="psS", tag="psS")
                for j in range(TCH):
                    nc.tensor.matmul(
                        psS[:, ds(off[j], widths[j])],
                        st["kT2"][:, h, ts(j, 128)],
                        st["qT"][:, pair, ds(128 * j, widths[j])],
                        start=True,
                        stop=True,
                    )
                nc.scalar.activation(aT, psS, AF.Exp, scale=SCALE)
                # zero the masked (s>t) part of the diagonal blocks (gpsimd)
                for j in range(TCH):
                    nc.gpsimd.tensor_mul(
                        aT[:, ds(off[j], 128)], aT[:, ds(off[j], 128)], mask_sb
                    )

            def av_head(b, h, aT, on_tiles):
                """attn @ [V | 1] for head h: four t-chunk groups accumulate
                into ONE psum bank [128, 4, 128]; merged reciprocal, then
                per-chunk per-partition scale into the on tiles."""
                st = state[b]
                pair, pb = h // 2, 64 * (h % 2)
                psAV = psB.tile([128, TCH, 128], f32, name="psAV", tag="psAV")
                for i in range(TCH):
                    for j in range(i + 1):
                        nc.tensor.matmul(
                            psAV[:, i, 0:65],
                            aT[:, ds(off[j] + 128 * (i - j), 128)],
                            st["v"][:, j, h, :],
                            start=(j == 0),
                            stop=(j == i),
                        )
                rr = spool.tile([128, TCH], f32, name="rr", tag="rr")
                nc.vector.reciprocal(rr, psAV[:, :, 64:65])
                for i in range(TCH):
                    nc.vector.tensor_scalar_mul(
                        on_tiles[i][:, ds(128 * pair + pb, 64)],
                        psAV[:, i, 0:64],
                        rr[:, i : i + 1],
                    )

            # ---------- batch 0 QKV (serial; DMA-order matched) ----------
            alloc_batch(0)
            for m in range(MO):
                q_chain(0, m)
            for m in range(MO):
                k_chain(0, m)
            for i in range(TCH):
                v_chain(0, i, 0)
            for i in range(TCH):
                v_chain(0, i, 1)

            # ---------- main loop: attention(b) fused with QKV(b+1) ----------
            held_back = {}
            for b in range(b_loc):
                nxt = b + 1 if b + 1 < b_loc else None
                fillers = []
                if nxt is not None:
                    alloc_batch(nxt)
                    issue_x_dma(nxt)
                    kc, q03, q47, v0, v1 = chain_pieces(nxt)
                    if nxt == b_loc - 1:
                        # last batch keeps its late chains as SELF-fillers for
                        # its own attention (nothing else hides exp latency)
                        fillers = kc + q03 + v0
                        held_back[nxt] = v1 + q47
                    else:
                        fillers = kc + q03 + v0 + q47 + v1
                fi = 0  # next filler index
                # self-fillers: V(i,1) consumed from head 8 (step 9), Q m4-7
                # from scores(8/10/12/14) — slots h=2..9 meet every deadline
                self_fill = held_back.pop(b, [])

                on_tiles = [
                    onpool.tile([128, HD], bf16, name=f"on{i}", tag=f"on{i}")
                    for i in range(TCH)
                ]
                # head-concat transposes, split in half: heads 0-7 (cols 0:512
                # of each on tile) are final after AV(7), so that half
                # transposes in the middle of the attention phase
                outT_sb = opool.tile([128, MO, T], bf16, name="outT_sb", tag="outT")
                aT_tiles = {}
                for h in range(H):
                    aT = apool.tile([128, PACK], bf16, name="aT", tag="aT")
                    aT_tiles[h] = aT
                    scores_head(b, h, aT)
                    if h >= 1:
                        av_head(b, h - 1, aT_tiles.pop(h - 1), on_tiles)
                    if h == 9:
                        for i in range(TCH):
                            nc.sync.dma_start_transpose(
                                out=outT_sb[:, 0:4, ts(i, 128)],
                                in_=on_tiles[i][:, 0:512],
                            )
                    if self_fill and 2 <= h <= 9:
                        self_fill[h - 2]()
                    elif h >= 2 and fi < len(fillers):
                        fillers[fi]()
                        fi += 1
                av_head(b, H - 1, aT_tiles.pop(H - 1), on_tiles)
                for _ in range(2):
                    if fi < len(fillers):
                        fillers[fi]()
                        fi += 1

                for i in range(TCH):
                    nc.sync.dma_start_transpose(
                        out=outT_sb[:, 4:8, ts(i, 128)],
                        in_=on_tiles[i][:, 512:1024],
                    )

                # ---- final projection; bias added during PSUM evacuation ----
                for i in range(TCH):
                    out_sb = opool.tile([128, C], f32, name="out_sb", tag="out_sb")
                    for half in range(2):
                        psF = psC.tile([128, 512], f32, name="psF", tag="ps1")
                        for k in range(MO):
                            nc.tensor.matmul(
                                psF,
                                outT_sb[:, k, ts(i, 128)],
                                wp_sb[:, k, ts(half, 512)],
                                start=(k == 0),
                                stop=(k == MO - 1),
                            )
                        nc.vector.tensor_add(
                            out=out_sb[:, ts(half, 512)],
                            in0=psF,
                            in1=bias_bc[:, ts(half, 512)],
                        )
                        if fi < len(fillers):
                            fillers[fi]()
                            fi += 1
                    nc.sync.dma_start(out=out[b, ts(i, 128), :], in_=out_sb)
                # any leftover fillers (shouldn't happen: 14+2+8 = 24)
                while fi < len(fillers):
                    fillers[fi]()
                    fi += 1

    nc.compile()
    return nc


def make_in_maps(x, wq, wk, wv, w_proj, b_proj, b_loc=B_LOC, ncores=NCORES):
    bf16 = ml_dtypes.bfloat16
    MO = HD // 128
    x = np.asarray(x, dtype=np.float32)
    # host-side layout prep (transpose / reshape / cast only)
    xT = np.ascontiguousarray(x.transpose(0, 2, 1)).astype(bf16)  # [B, C, T]
    wq2 = np.asarray(wq, np.float32).transpose(1, 0, 2).reshape(C, HD)
    wk2 = np.asarray(wk, np.float32).transpose(1, 0, 2).reshape(C, HD)
    # m-major: [MO, C, 128]
    wqm = np.ascontiguousarray(wq2.reshape(C, MO, 128).transpose(1, 0, 2)).astype(bf16)
    wkm = np.ascontiguousarray(wk2.reshape(C, MO, 128).transpose(1, 0, 2)).astype(bf16)
    wv2 = np.ascontiguousarray(
        np.asarray(wv, np.float32).transpose(1, 0, 2).reshape(C, HD)
    ).astype(bf16)
    wp2 = np.ascontiguousarray(np.asarray(w_proj, np.float32)).astype(bf16)
    bp2 = np.asarray(b_proj, np.float32).reshape(1, C).astype(bf16)
    # mask[p, f] = 1 where p <= f (valid: s_in <= t_in on diagonal blocks)
    m = np.triu(np.ones((128, 128), np.float32)).astype(bf16)
    in_maps = []
    for c in range(ncores):
        in_maps.append(
            {
                "xT": xT[c * b_loc : (c + 1) * b_loc],
                "wqm": wqm,
                "wkm": wkm,
                "wv": wv2,
                "wp": wp2,
                "bp": bp2,
                "mask": m,
            }
        )
    return in_maps


def kernel(x, wq, wk, wv, w_proj, b_proj, **run_kwargs):
    from concourse import bass_utils

    if "nc" not in _CACHE:
        _CACHE["nc"] = build_nc(B_LOC)
    nc = _CACHE["nc"]
    in_maps = make_in_maps(x, wq, wk, wv, w_proj, b_proj)
    res = bass_utils.run_bass_kernel_spmd(
        nc, in_maps, core_ids=list(range(NCORES)), **run_kwargs
    )
    outs = [r["out"] for r in res.results]
    full = np.concatenate(outs, axis=0).astype(np.float32)
    if run_kwargs:
        _CACHE["last_result"] = res
    return full
